# revision 18
# baseline (speedup 1.0000x reference)
"""Sparse cross-attention kernel for TRN2 (8 NeuronCores, SPMD data-parallel over batch).

Reference computation (per batch b):
    enc_q = enc @ Wq.T + bq; enc_v = enc @ Wv.T + bv; dec_q = Wd @ h + bd
    energy = tanh(enc_q @ dec_q); w = softmax(energy); out = w @ enc_v

Algebraic rewrite (exact, avoids materializing enc_q / enc_v):
    r[b] = Wq.T (Wd h_b + bd)               [E]   (tiny: computed on host, like
    c[b] = bq . (Wd h_b + bd)               scalar the host-fused GT=Wd.T Wq)
    energy[l] = enc[l,:] . r + c
    t    = tanh(energy) in [-1,1]  ->  exp() safe without max subtraction
    wexp = exp(t);  Z = sum_l wexp;  s~ = sum_l wexp[l] * enc[l,:]
    out  = (s~ @ Wv.T) / Z + bv             (1/Z + bias folded into one STT)

Turns a 210 GFLOP problem into a bf16 streaming problem bound by reading
encoder_outputs once (12.8 MB/core) + Wv.T (4 MB).

Device mapping per core (BLOC=16 batches, L split 128+68):
  The energy dot (mult + free-dim reduce over E) is split across engines by
  e-quarter units of [*,512]:
    DVE:   units (q0,lt0/1), (q1,lt0/1), (q2,lt0) as fused
           scalar_tensor_tensor with free-dim accumulate, reading the
           r-broadcast from PSUM (PE ones-matmul, produced a batch ahead).
    Pool:  units (q2,lt1), (q3,lt0/1) as tensor_tensor multiplies reading a
           DMA-broadcast r tile (HBM row -> 128 partitions, stride-0 AP).
    ACT:   reduces Pool's products via activation-accumulate; tanh; exp.
  PE:    r partition-broadcasts (PSUM, quarters 0-2), s~/Z accumulation via
         zero-padded-column lhsT (Z shares s's PSUM banks through a
         col-group-32 tile_position), s~ transposes, context matmuls.
  Tail:  drain s~ (ACT, chunked), transpose, context matmul vs Wv.T, then one
         STT fuses the 1/Z scale and +bv bias; single store.
"""

import numpy as np
import ml_dtypes

import concourse.bass as bass
import concourse.mybir as mybir
from concourse import bacc
from concourse.bass import ds
from concourse.tile import TileContext
from concourse.bass_utils import run_bass_kernel_spmd
from concourse._compat import with_exitstack

BF16 = mybir.dt.bfloat16
F32 = mybir.dt.float32

B, L, E, D, A = 128, 196, 2048, 1024, 1024
NCORES = 8
BLOC = B // NCORES          # 16 batches per core
EJ = E // 128               # 16 e-chunks of 128 (transposes / context)
NL0, NL1 = 128, L - 128     # l-tile sizes: 128 + 68


def _bcast(ap_row, parts):
    """AP reading a row-shaped DRAM slice broadcast across `parts` partitions."""
    inner = [list(x) for x in ap_row.ap]
    if len(inner) > 1 and inner[0][1] == 1:
        inner = inner[1:]
    return bass.AP(tensor=ap_row.tensor, offset=ap_row.offset,
                   ap=[[0, parts]] + inner)


@with_exitstack
def _body(ctx, tc, enc, rc, crow, wvt, bv, identity, out):
    nc = tc.nc
    AF = mybir.ActivationFunctionType
    OP = mybir.AluOpType

    consts = ctx.enter_context(tc.tile_pool(name="consts", bufs=1))

    ident = consts.tile([128, 128], BF16)
    ones_row = consts.tile([1, 128], BF16)    # lhsT for partition-broadcast matmul
    nc.vector.memset(ones_row[:, :], 1.0)
    ones_col = consts.tile([128, 1], BF16)    # rhs for the Z matmul
    nc.vector.memset(ones_col[:, :], 1.0)

    # r rows flattened onto partition 0 (matmul rhs must be base-partition 0)
    rc_flat = consts.tile([1, BLOC * E], BF16)
    nc.sync.dma_start(out=rc_flat[:, :], in_=rc[:, :])
    rcv = rc_flat.rearrange("p (b e) -> p b e", e=E)
    bv_rep = consts.tile([BLOC, A], F32)
    c_rep = consts.tile([128, BLOC], F32)
    # all-batch wexp lhsT: zeroed once; batch b writes its exp into col b
    wexp_all = consts.tile([128, BLOC, 2, BLOC], BF16)
    nc.vector.memset(wexp_all[:, :, :, :], 0.0)
    wvt_sb = consts.tile([128, EJ, A], BF16)

    # ---- loop pools ----
    rrep_ps_cm = tc.tile_pool(name="rrep_ps", bufs=4, space="PSUM")
    rrep_ps_pool = rrep_ps_cm.__enter__()
    rq_pool = ctx.enter_context(tc.tile_pool(name="rqp", bufs=3))
    enc_pool = ctx.enter_context(tc.tile_pool(name="encp", bufs=12))
    work = ctx.enter_context(tc.tile_pool(name="work", bufs=2))
    scr_pool = ctx.enter_context(tc.tile_pool(name="scr", bufs=3))
    prod_pool = ctx.enter_context(tc.tile_pool(name="prod", bufs=3))
    epi = ctx.enter_context(tc.tile_pool(name="epi", bufs=1))

    loop_psum_cm = tc.tile_pool(name="loop_psum", bufs=1, space="PSUM")
    loop_psum = loop_psum_cm.__enter__()
    # s~ on partitions 0-15 and Z on partitions 32-47 share the same 4 banks
    sz_psum = loop_psum.tile([48, E], F32)
    s_psum = sz_psum[0:BLOC, :]
    z_psum = sz_psum[32:32 + BLOC, 0:1]

    def produce_rrep(b):
        """r[b, quarters 0-2] broadcast to PSUM via PE ones-matmuls (for the
        DVE STT units) + quarters 2-3 broadcast to SBUF bf16 via DMA (for the
        Pool tensor_tensor units; DVE DGE queue keeps SP free for enc)."""
        rps = []
        for q in range(3):
            rp = rrep_ps_pool.tile([128, 512], F32, tag="rrep_ps",
                                   name=f"rp{b}_{q}")
            nc.tensor.matmul(rp[:, :], ones_row[:, :],
                             rcv[0:1, b, ds(q * 512, 512)],
                             start=True, stop=True)
            rps.append(rp)
        rq = rq_pool.tile([128, 1024], BF16, tag="rq", name=f"rq{b}")
        nc.sync.dma_start(out=rq[:NL1, 0:512],
                          in_=_bcast(rc[b:b + 1, ds(E // 2, 512)], NL1))
        nc.sync.dma_start(out=rq[:, 512:1024],
                          in_=_bcast(rc[b:b + 1, ds(3 * E // 4, 512)], 128))
        return rps, rq

    def load_enc(b):
        et0 = enc_pool.tile([128, E], BF16, tag="enc", name=f"et0_{b}")
        nc.sync.dma_start(out=et0[:, :], in_=enc[b, 0:NL0, :])
        et1 = enc_pool.tile([128, E], BF16, tag="enc", name=f"et1_{b}")
        nc.sync.dma_start(out=et1[:NL1, :], in_=enc[b, NL0:L, :])
        return [(et0, NL0), (et1, NL1)]

    PF = 3   # enc prefetch depth (batches)
    # startup order matters: enc b0/b1 first on the SP queue; everything else
    # paced via tile_wait_until so dep-free DMAs can't starve the enc stream
    et_tiles = {0: load_enc(0)}
    rrep_tiles = {0: produce_rrep(0)}
    nc.sync.dma_start(out=c_rep[:, :], in_=_bcast(crow[0:1, :], 128))
    et_tiles[1] = load_enc(1)
    rrep_tiles[1] = produce_rrep(1)
    wvtv = wvt[:, :].rearrange("(j p) a -> p j a", p=128)
    et_tiles[2] = load_enc(2)
    pending = {}   # software pipeline: batch b's softmax+accumulate stage is
                   # emitted after batch b+1's energy stage

    def energy_stage(b):
        if b + 1 < BLOC and b + 1 not in rrep_tiles:
            rrep_tiles[b + 1] = produce_rrep(b + 1)
        if b + PF < BLOC:
            et_tiles[b + PF] = load_enc(b + PF)
        # one wvt chunk per batch on the same in-order SP queue, emitted
        # after the enc prefetch so it never delays the enc stream
        nc.sync.dma_start(out=wvt_sb[:, ds(b, 1), :],
                          in_=wvtv[:, ds(b, 1), :])
        if b == 1:
            nc.sync.dma_start(out=ident[:, :], in_=identity[:, :])
        if b == 2:
            nc.sync.dma_start(out=bv_rep[:, :], in_=_bcast(bv[:], BLOC))
        rrep_ps_b, rq_b = rrep_tiles.pop(b)
        ets = et_tiles.pop(b)

        # energy partial sums: separate tiles per l-tile so consecutive
        # units never touch the same tile (avoids false WAW serialization)
        esum0 = work.tile([128, 4], F32, tag="esum0", name=f"esum0_{b}")
        esum1 = work.tile([128, 4], F32, tag="esum1", name=f"esum1_{b}")
        esums = (esum0, esum1)
        # DVE fused units; for the last batch take the lt1 Pool units onto
        # DVE too (shorter tail critical path through Pool->ACT)
        dve_units = [(0, 0), (0, 1), (1, 0), (1, 1), (2, 0)]
        pool_units = [(2, 1), (3, 0), (3, 1)]
        if b == BLOC - 1:
            dve_units += [(2, 1), (3, 1)]
            pool_units = [(3, 0)]
        for q, lt in dve_units:
            et, nl = ets[lt]
            scr = scr_pool.tile([128, 512], BF16, tag="scr",
                                name=f"scr{b}_{q}{lt}")
            in1 = (rrep_ps_b[q][:nl, :] if q < 3
                   else rq_b[:nl, ds((q - 2) * 512, 512)])
            nc.vector.scalar_tensor_tensor(
                out=scr[:nl, :],
                in0=et[:nl, ds(q * 512, 512)],
                scalar=0.0,
                in1=in1,
                op0=OP.bypass,
                op1=OP.mult,
                accum_out=esums[lt][:nl, ds(q, 1)],
            )
        # Pool multiplies (ACT reduces emitted separately, after the previous
        # batch's softmax, so tanh/exp never queue behind them on ACT)
        prods = []
        for q, lt in pool_units:
            et, nl = ets[lt]
            prod = prod_pool.tile([128, 512], BF16, tag="prod",
                                  name=f"prod{b}_{q}{lt}")
            nc.gpsimd.tensor_tensor(
                out=prod[:nl, :],
                in0=et[:nl, ds(q * 512, 512)],
                in1=rq_b[:nl, ds((q - 2) * 512, 512)],
                op=OP.mult)
            prods.append((q, lt, nl, prod))
        pending[b] = (ets, esums, prods)

    def act_reduces(b):
        ets, esums, prods = pending[b]
        for q, lt, nl, prod in prods:
            scr = scr_pool.tile([128, 512], BF16, tag="ascr",
                                name=f"ascr{b}_{q}{lt}")
            nc.scalar.activation(out=scr[:nl, :], in_=prod[:nl, :],
                                 func=AF.Copy,
                                 accum_out=esums[lt][:nl, ds(q, 1)])

    def softmax_stage(b):
        if b >= BLOC - 2:
            # keep PE continuously busy so the tail matmuls run at full clock
            for w in range(4):
                fp = rrep_ps_pool.tile([128, 512], F32, tag="rrep_ps",
                                       name=f"warm{b}_{w}")
                nc.tensor.matmul(fp[:, :], ones_row[:, :],
                                 rcv[0:1, BLOC - 1, ds((w % 4) * 512, 512)],
                                 start=True, stop=True)
        ets, esums, _ = pending.pop(b)
        en = work.tile([128, 2], F32, tag="en", name=f"en{b}")
        for lt in range(2):
            nc.vector.tensor_reduce(en[:, ds(lt, 1)], esums[lt][:, :],
                                    axis=mybir.AxisListType.X, op=OP.add)
        tcol = work.tile([128, 2], F32, tag="tcol", name=f"tcol{b}")
        nc.scalar.activation(out=tcol[:, :], in_=en[:, :], func=AF.Tanh,
                             bias=c_rep[:, ds(b, 1)], scale=1.0)
        nc.scalar.activation(out=wexp_all[:, b, :, b], in_=tcol[:, :], func=AF.Exp)

        # s~ and Z accumulation across all batches (row b via zero-padded col b)
        for lt, (et, nl) in enumerate(ets):
            first = (b == 0 and lt == 0)
            last = (b == BLOC - 1 and lt == 1)
            wl = wexp_all[:nl, b, lt, :]
            nc.tensor.matmul(z_psum[:, :], wl, ones_col[:nl, :],
                             start=first, stop=last, tile_position=(0, 32))
            for j in range(4):
                nc.tensor.matmul(
                    s_psum[:, ds(j * 512, 512)],
                    wl,
                    et[:nl, ds(j * 512, 512)],
                    start=first, stop=last,
                )

    for b in range(BLOC):
        energy_stage(b)
        if b >= 1:
            softmax_stage(b - 1)
        act_reduces(b)
    softmax_stage(BLOC - 1)

    # ---- epilogue: context = (s~ @ Wv.T) / Z + bv ----
    # drains paired DVE/ACT; transposes + copies + ctx interleaved per chunk
    # in drain-readiness order so PE rolls straight into the ctx matmuls
    s_sb = epi.tile([BLOC, E], BF16)
    nc.vector.tensor_copy(s_sb[:, :], s_psum[:, :])
    z_sb = epi.tile([BLOC, 1], F32)
    nc.vector.tensor_copy(z_sb[:, :], z_psum[:, :])
    zinv = epi.tile([BLOC, 1], F32)
    nc.vector.reciprocal(zinv[:, :], z_sb[:, :])
    loop_psum_cm.__exit__(None, None, None)
    rrep_ps_cm.__exit__(None, None, None)   # free banks for the epilogue

    sTs = [epi.tile([128, BLOC], BF16, name=f"sT{j}") for j in range(EJ)]
    cpool_cm = tc.tile_pool(name="ctx_psum", bufs=1, space="PSUM")
    cpool = cpool_cm.__enter__()
    ctx_ps = cpool.tile([BLOC, A], F32)
    tp_cm = tc.tile_pool(name="tp_psum", bufs=6, space="PSUM")
    tp_pool = tp_cm.__enter__()
    for j in range(EJ):
        tp = tp_pool.tile([128, BLOC], BF16, tag="tp")
        nc.tensor.transpose(tp[:, :], s_sb[:, ds(j * 128, 128)],
                            ident[:BLOC, :BLOC])
        if j % 2 == 1:
            nc.scalar.copy(sTs[j][:, :], tp[:, :])
        else:
            nc.vector.tensor_copy(sTs[j][:, :], tp[:, :])
    for j in range(EJ):
        for a2 in range(A // 512):
            nc.tensor.matmul(
                ctx_ps[:, ds(a2 * 512, 512)],
                sTs[j][:, :],
                wvt_sb[:, j, ds(a2 * 512, 512)],
                start=(j == 0), stop=(j == EJ - 1),
            )
    tp_cm.__exit__(None, None, None)
    if True:
        ctx_sb = epi.tile([BLOC, A], F32)
        # fused: out = ctx * (1/Z) + bv
        nc.vector.scalar_tensor_tensor(
            out=ctx_sb[:, :], in0=ctx_ps[:, :], scalar=zinv[:, 0:1],
            in1=bv_rep[:, :], op0=OP.mult, op1=OP.add)
        nc.sync.dma_start(out=out[:, :], in_=ctx_sb[:, :])
    cpool_cm.__exit__(None, None, None)


def _build():
    nc = bacc.Bacc()
    enc = nc.dram_tensor("enc", [BLOC, L, E], BF16, kind="ExternalInput")
    rc = nc.dram_tensor("rc", [BLOC, E], BF16, kind="ExternalInput")
    crow = nc.dram_tensor("crow", [1, BLOC], F32, kind="ExternalInput")
    wvt = nc.dram_tensor("wvt", [E, A], BF16, kind="ExternalInput")
    bv = nc.dram_tensor("bv", [A], F32, kind="ExternalInput")
    identity = nc.dram_tensor("identity", [128, 128], BF16, kind="ExternalInput")
    out = nc.dram_tensor("out", [BLOC, A], F32, kind="ExternalOutput")

    with TileContext(nc, pool_alloc_mode="queue") as tc:
        _body(tc, enc, rc, crow, wvt, bv, identity, out)
    nc.finalize()
    return nc


_CACHE = {}


def _nc():
    if "nc" not in _CACHE:
        _CACHE["nc"] = _build()
    return _CACHE["nc"]


def _prep(encoder_outputs, decoder_hidden, Wq, bq, Wv, bv, Wd, bd):
    bf = ml_dtypes.bfloat16
    enc = np.ascontiguousarray(np.asarray(encoder_outputs, dtype=np.float32))
    h = np.asarray(decoder_hidden, dtype=np.float32)
    Wq = np.asarray(Wq, dtype=np.float32)
    bq = np.asarray(bq, dtype=np.float32)
    Wv = np.asarray(Wv, dtype=np.float32)
    bv = np.ascontiguousarray(np.asarray(bv, dtype=np.float32))
    Wd = np.asarray(Wd, dtype=np.float32)
    bd = np.asarray(bd, dtype=np.float32)

    # host-fused small projections (0.25% of the FLOPs):
    #   r = Wq.T (Wd h + bd) per batch; c = bq . (Wd h + bd)
    dec_q = h @ Wd.T + bd                   # [B, A]
    rv = dec_q @ Wq                         # [B, E]
    cv = dec_q @ bq                         # [B]

    wvt_b = np.ascontiguousarray(Wv.T.astype(bf))
    enc_b = enc.astype(bf)
    ident = np.ascontiguousarray(np.eye(128, dtype=np.float32).astype(bf))

    in_maps = []
    for i in range(NCORES):
        sl = slice(i * BLOC, (i + 1) * BLOC)
        in_maps.append({
            "enc": np.ascontiguousarray(enc_b[sl]),
            "rc": np.ascontiguousarray(rv[sl].astype(bf)),
            "crow": np.ascontiguousarray(cv[sl].reshape(1, BLOC)),
            "wvt": wvt_b,
            "bv": bv,
            "identity": ident,
        })
    return in_maps


def run(inputs, trace=False):
    in_maps = _prep(**inputs)
    res = run_bass_kernel_spmd(_nc(), in_maps, core_ids=list(range(NCORES)),
                               trace=trace)
    out = np.concatenate([r["out"] for r in res.results], axis=0).astype(np.float32)
    return out, res.exec_time_ns


def kernel(**inputs):
    out, _ = run(inputs, trace=False)
    return out


# revision 26
# speedup vs baseline: 1.0550x; 1.0550x over previous
"""Sparse cross-attention kernel for TRN2 (8 NeuronCores, SPMD data-parallel over batch).

Reference computation (per batch b):
    enc_q = enc @ Wq.T + bq; enc_v = enc @ Wv.T + bv; dec_q = Wd @ h + bd
    energy = tanh(enc_q @ dec_q); w = softmax(energy); out = w @ enc_v

Algebraic rewrite (exact, avoids materializing enc_q / enc_v):
    r[b] = Wq.T (Wd h_b + bd)               [E]   (tiny: computed on host, like
    c[b] = bq . (Wd h_b + bd)               scalar the host-fused GT=Wd.T Wq)
    energy[l] = enc[l,:] . r + c
    t    = tanh(energy) in [-1,1]  ->  exp() safe without max subtraction
    wexp = exp(t);  Z = sum_l wexp;  s~ = sum_l wexp[l] * enc[l,:]
    out  = (s~ @ Wv.T) / Z + bv             (1/Z + bias folded into one STT)

Turns a 210 GFLOP problem into a bf16 streaming problem bound by reading
encoder_outputs once (12.8 MB/core) + Wv.T (4 MB).

Device mapping per core (BLOC=16 batches, L split 128+68):
  The energy dot (mult + free-dim reduce over E) is split across engines by
  e-quarter units of [*,512]:
    DVE:   units (q0,lt0/1), (q1,lt0/1), (q2,lt0) as fused
           scalar_tensor_tensor with free-dim accumulate, reading the
           r-broadcast from PSUM (PE ones-matmul, produced a batch ahead).
    Pool:  units (q2,lt1), (q3,lt0/1) as tensor_tensor multiplies reading a
           DMA-broadcast r tile (HBM row -> 128 partitions, stride-0 AP).
    ACT:   reduces Pool's products via activation-accumulate; tanh; exp.
  PE:    r partition-broadcasts (PSUM, quarters 0-2), s~/Z accumulation via
         zero-padded-column lhsT (Z shares s's PSUM banks through a
         col-group-32 tile_position), s~ transposes, context matmuls.
  Tail:  drain s~ (ACT, chunked), transpose, context matmul vs Wv.T, then one
         STT fuses the 1/Z scale and +bv bias; single store.
"""

import numpy as np
import ml_dtypes

import concourse.bass as bass
import concourse.mybir as mybir
from concourse import bacc
from concourse.bass import ds
from concourse.tile import TileContext
from concourse.bass_utils import run_bass_kernel_spmd
from concourse._compat import with_exitstack

BF16 = mybir.dt.bfloat16
F32 = mybir.dt.float32

B, L, E, D, A = 128, 196, 2048, 1024, 1024
NCORES = 8
BLOC = B // NCORES          # 16 batches per core
EJ = E // 128               # 16 e-chunks of 128 (transposes / context)
NL0, NL1 = 128, L - 128     # l-tile sizes: 128 + 68


def _bcast(ap_row, parts):
    """AP reading a row-shaped DRAM slice broadcast across `parts` partitions."""
    inner = [list(x) for x in ap_row.ap]
    if len(inner) > 1 and inner[0][1] == 1:
        inner = inner[1:]
    return bass.AP(tensor=ap_row.tensor, offset=ap_row.offset,
                   ap=[[0, parts]] + inner)


@with_exitstack
def _body(ctx, tc, enc, rc, crow, wvt, bv, identity, out):
    nc = tc.nc
    AF = mybir.ActivationFunctionType
    OP = mybir.AluOpType

    consts = ctx.enter_context(tc.tile_pool(name="consts", bufs=1))

    ident = consts.tile([128, 128], BF16)
    ones_row = consts.tile([1, 128], BF16)    # lhsT for partition-broadcast matmul
    nc.vector.memset(ones_row[:, :], 1.0)
    ones_col = consts.tile([128, 1], BF16)    # rhs for the Z matmul
    nc.vector.memset(ones_col[:, :], 1.0)

    # r rows flattened onto partition 0 (matmul rhs must be base-partition 0)
    rc_flat = consts.tile([1, BLOC * E], BF16)
    nc.sync.dma_start(out=rc_flat[:, :], in_=rc[:, :])
    rcv = rc_flat.rearrange("p (b e) -> p b e", e=E)
    bv_rep = consts.tile([BLOC, A], F32)
    c_rep = consts.tile([128, BLOC], F32)
    # all-batch wexp lhsT: zeroed once; batch b writes its exp into col b
    wexp_all = consts.tile([128, BLOC, 2, BLOC], BF16)
    nc.vector.memset(wexp_all[:, :, :, :], 0.0)
    wvt_sb = consts.tile([128, EJ, A], BF16)

    # ---- loop pools ----
    rrep_ps_cm = tc.tile_pool(name="rrep_ps", bufs=4, space="PSUM")
    rrep_ps_pool = rrep_ps_cm.__enter__()
    rq_pool = ctx.enter_context(tc.tile_pool(name="rqp", bufs=3))
    enc_pool = ctx.enter_context(tc.tile_pool(name="encp", bufs=12))
    work = ctx.enter_context(tc.tile_pool(name="work", bufs=2))
    scr_pool = ctx.enter_context(tc.tile_pool(name="scr", bufs=6))
    prod_pool = ctx.enter_context(tc.tile_pool(name="prod", bufs=4))
    epi = ctx.enter_context(tc.tile_pool(name="epi", bufs=1))

    loop_psum_cm = tc.tile_pool(name="loop_psum", bufs=1, space="PSUM")
    loop_psum = loop_psum_cm.__enter__()
    # s~ split into two tiles so the two epilogue drains can run in
    # parallel; Z shares tile A's banks via the col-group-32 tile_position
    sz_psumA = loop_psum.tile([48, E // 2], F32)
    sz_psumB = loop_psum.tile([BLOC, E // 2], F32)
    s_psumA = sz_psumA[0:BLOC, :]
    s_psumB = sz_psumB[:, :]
    z_psum = sz_psumA[32:32 + BLOC, 0:1]

    def produce_rrep(b):
        """r[b, quarters 0-2] broadcast to PSUM via PE ones-matmuls (for the
        DVE STT units) + quarters 2-3 broadcast to SBUF bf16 via DMA (for the
        Pool tensor_tensor units; DVE DGE queue keeps SP free for enc)."""
        rps = []
        for q in range(3):
            rp = rrep_ps_pool.tile([128, 512], F32, tag="rrep_ps",
                                   name=f"rp{b}_{q}")
            nc.tensor.matmul(rp[:, :], ones_row[:, :],
                             rcv[0:1, b, ds(q * 512, 512)],
                             start=True, stop=True)
            rps.append(rp)
        rq = rq_pool.tile([128, 1024], BF16, tag="rq", name=f"rq{b}")
        nc.sync.dma_start(out=rq[:NL1, 0:512],
                          in_=_bcast(rc[b:b + 1, ds(E // 2, 512)], NL1))
        nc.sync.dma_start(out=rq[:, 512:1024],
                          in_=_bcast(rc[b:b + 1, ds(3 * E // 4, 512)], 128))
        return rps, rq

    def load_enc(b):
        et0 = enc_pool.tile([128, E], BF16, tag="enc", name=f"et0_{b}")
        nc.sync.dma_start(out=et0[:, :], in_=enc[b, 0:NL0, :])
        et1 = enc_pool.tile([128, E], BF16, tag="enc", name=f"et1_{b}")
        nc.sync.dma_start(out=et1[:NL1, :], in_=enc[b, NL0:L, :])
        return [(et0, NL0), (et1, NL1)]

    PF = 3   # enc prefetch depth (batches)
    # startup order matters: enc b0/b1 first on the SP queue; everything else
    # paced via tile_wait_until so dep-free DMAs can't starve the enc stream
    et_tiles = {0: load_enc(0)}
    rrep_tiles = {0: produce_rrep(0)}
    nc.sync.dma_start(out=c_rep[:, :], in_=_bcast(crow[0:1, :], 128))
    et_tiles[1] = load_enc(1)
    rrep_tiles[1] = produce_rrep(1)
    wvtv = wvt[:, :].rearrange("(j p) a -> p j a", p=128)
    et_tiles[2] = load_enc(2)
    pending = {}   # software pipeline: batch b's softmax+accumulate stage is
                   # emitted after batch b+1's energy stage

    def energy_stage(b):
        if b + 1 < BLOC and b + 1 not in rrep_tiles:
            rrep_tiles[b + 1] = produce_rrep(b + 1)
        if b + PF < BLOC:
            et_tiles[b + PF] = load_enc(b + PF)
        # one wvt chunk per batch on the same in-order SP queue, emitted
        # after the enc prefetch so it never delays the enc stream
        nc.sync.dma_start(out=wvt_sb[:, ds(b, 1), :],
                          in_=wvtv[:, ds(b, 1), :])
        if b == 1:
            nc.sync.dma_start(out=ident[:, :], in_=identity[:, :])
        if b == 2:
            nc.sync.dma_start(out=bv_rep[:, :], in_=_bcast(bv[:], BLOC))
        rrep_ps_b, rq_b = rrep_tiles.pop(b)
        ets = et_tiles.pop(b)

        # energy partial sums: separate tiles per l-tile so consecutive
        # units never touch the same tile (avoids false WAW serialization)
        esum0 = work.tile([128, 4], F32, tag="esum0", name=f"esum0_{b}")
        esum1 = work.tile([128, 5], F32, tag="esum1", name=f"esum1_{b}")
        esums = (esum0, esum1)
        # DVE fused units; for the last batch take the lt1 Pool units onto
        # DVE too (shorter tail critical path through Pool->ACT)
        dve_units = [(0, 0), (0, 1), (1, 0), (1, 1), (2, 0)]
        # (2, 1) with width 1024 = merged quarters 2+3 of l-tile 1 (their
        # partial sums share one esum slot; only the total matters)
        pool_units = [(2, 1, 1024), (3, 0, 512)]
        if b == BLOC - 1:
            dve_units += [(2, 1)]
            pool_units = [(3, 0, 512), (3, 1, 512)]
        elif b == 0:
            # all-DVE first batch: the softmax chain never waits on the cold
            # Pool->ACT path, so PE's s-matmuls and the next rreps flow early
            dve_units += [(2, 1), (3, 0), (3, 1)]
            pool_units = []
        SLIV = 128   # leading cols of (2,1) handled by DVE, rest by Pool
        for q, lt in dve_units:
            et, nl = ets[lt]
            scr = scr_pool.tile([128, 512], BF16, tag="scr",
                                name=f"scr{b}_{q}{lt}")
            in1 = (rrep_ps_b[q][:nl, :] if q < 3
                   else rq_b[:nl, ds((q - 2) * 512, 512)])
            nc.vector.scalar_tensor_tensor(
                out=scr[:nl, :],
                in0=et[:nl, ds(q * 512, 512)],
                scalar=0.0,
                in1=in1,
                op0=OP.bypass,
                op1=OP.mult,
                accum_out=esums[lt][:nl, ds(q, 1)],
            )
        # Pool multiplies (ACT reduces emitted separately, after the previous
        # batch's softmax, so tanh/exp never queue behind them on ACT)
        prods = []
        for q, lt, w in pool_units:
            et, nl = ets[lt]
            prod = prod_pool.tile([128, 1024], BF16, tag="prod",
                                  name=f"prod{b}_{q}{lt}")
            nc.gpsimd.tensor_tensor(
                out=prod[:nl, 0:w],
                in0=et[:nl, ds(q * 512, w)],
                in1=rq_b[:nl, ds((q - 2) * 512, w)],
                op=OP.mult)
            prods.append((q, lt, nl, w, prod))
        pending[b] = (ets, esums, prods)

    def act_reduces(b):
        ets, esums, prods = pending[b]
        for q, lt, nl, w, prod in prods:
            scr = scr_pool.tile([128, 1024], BF16, tag="ascr",
                                name=f"ascr{b}_{q}{lt}")
            nc.scalar.activation(out=scr[:nl, 0:w], in_=prod[:nl, 0:w],
                                 func=AF.Copy,
                                 accum_out=esums[lt][:nl, ds(q, 1)])

    def softmax_stage(b):
        if b >= BLOC - 2:
            # keep PE continuously busy so the tail matmuls run at full clock
            for w in range(4):
                fp = rrep_ps_pool.tile([128, 512], F32, tag="rrep_ps",
                                       name=f"warm{b}_{w}")
                nc.tensor.matmul(fp[:, :], ones_row[:, :],
                                 rcv[0:1, BLOC - 1, ds((w % 4) * 512, 512)],
                                 start=True, stop=True)
        ets, esums, prods = pending.pop(b)
        en = work.tile([128, 2], F32, tag="en", name=f"en{b}")
        enscr = work.tile([128, 2, 4], F32, tag="enscr", name=f"enscr{b}")
        for lt in range(2):
            # lt1 of steady batches has quarters 2+3 merged into slot 2;
            # slot 3 is unwritten there and must not be summed
            w = 3 if (lt == 1 and 0 < b < BLOC - 1) else 4
            nc.scalar.activation(out=enscr[:, lt, 0:w], in_=esums[lt][:, 0:w],
                                 func=AF.Copy, accum_out=en[:, ds(lt, 1)])
        tcol = work.tile([128, 2], F32, tag="tcol", name=f"tcol{b}")
        nc.scalar.activation(out=tcol[:, :], in_=en[:, :], func=AF.Tanh,
                             bias=c_rep[:, ds(b, 1)], scale=1.0)
        nc.scalar.activation(out=wexp_all[:, b, :, b], in_=tcol[:, :], func=AF.Exp)

        # s~ and Z accumulation across all batches (row b via zero-padded col b)
        for lt, (et, nl) in enumerate(ets):
            first = (b == 0 and lt == 0)
            last = (b == BLOC - 1 and lt == 1)
            wl = wexp_all[:nl, b, lt, :]
            nc.tensor.matmul(z_psum[:, :], wl, ones_col[:nl, :],
                             start=first, stop=last, tile_position=(0, 32))
            for j in range(4):
                dst = (s_psumA[:, ds(j * 512, 512)] if j < 2
                       else s_psumB[:, ds((j - 2) * 512, 512)])
                nc.tensor.matmul(
                    dst,
                    wl,
                    et[:nl, ds(j * 512, 512)],
                    start=first, stop=last,
                )

    for b in range(BLOC):
        energy_stage(b)
        if b >= 1:
            softmax_stage(b - 1)
        act_reduces(b)
    softmax_stage(BLOC - 1)

    # ---- epilogue: context = (s~ @ Wv.T) / Z + bv ----
    # drains paired DVE/ACT; transposes + copies + ctx interleaved per chunk
    # in drain-readiness order so PE rolls straight into the ctx matmuls
    s_sb = epi.tile([BLOC, E], BF16)
    nc.vector.tensor_copy(s_sb[:, 0:1024], s_psumA[:, :])
    nc.scalar.activation(out=s_sb[:, 1024:2048], in_=s_psumB[:, :],
                         func=AF.Copy)
    z_sb = epi.tile([BLOC, 1], F32)
    nc.vector.tensor_copy(z_sb[:, :], z_psum[:, :])
    zinv = epi.tile([BLOC, 1], F32)
    nc.vector.reciprocal(zinv[:, :], z_sb[:, :])
    loop_psum_cm.__exit__(None, None, None)
    rrep_ps_cm.__exit__(None, None, None)   # free banks for the epilogue

    sTs = [epi.tile([128, BLOC], BF16, name=f"sT{j}") for j in range(EJ)]
    cpool_cm = tc.tile_pool(name="ctx_psum", bufs=1, space="PSUM")
    cpool = cpool_cm.__enter__()
    ctx_ps = cpool.tile([BLOC, A], F32)
    tp_cm = tc.tile_pool(name="tp_psum", bufs=6, space="PSUM")
    tp_pool = tp_cm.__enter__()
    for j in range(EJ):
        tp = tp_pool.tile([128, BLOC], BF16, tag="tp")
        nc.tensor.transpose(tp[:, :], s_sb[:, ds(j * 128, 128)],
                            ident[:BLOC, :BLOC])
        if j % 2 == 1:
            nc.scalar.copy(sTs[j][:, :], tp[:, :])
        else:
            nc.vector.tensor_copy(sTs[j][:, :], tp[:, :])
    ctx_sb = epi.tile([BLOC, A], F32)
    for a2 in range(A // 512):
        for j in range(EJ):
            nc.tensor.matmul(
                ctx_ps[:, ds(a2 * 512, 512)],
                sTs[j][:, :],
                wvt_sb[:, j, ds(a2 * 512, 512)],
                start=(j == 0), stop=(j == EJ - 1),
            )
        # fused normalize+bias and store for this half while PE continues
        nc.vector.scalar_tensor_tensor(
            out=ctx_sb[:, ds(a2 * 512, 512)], in0=ctx_ps[:, ds(a2 * 512, 512)],
            scalar=zinv[:, 0:1], in1=bv_rep[:, ds(a2 * 512, 512)],
            op0=OP.mult, op1=OP.add)
        nc.sync.dma_start(out=out[:, ds(a2 * 512, 512)],
                          in_=ctx_sb[:, ds(a2 * 512, 512)])
    tp_cm.__exit__(None, None, None)
    cpool_cm.__exit__(None, None, None)


def _build():
    nc = bacc.Bacc()
    enc = nc.dram_tensor("enc", [BLOC, L, E], BF16, kind="ExternalInput")
    rc = nc.dram_tensor("rc", [BLOC, E], BF16, kind="ExternalInput")
    crow = nc.dram_tensor("crow", [1, BLOC], F32, kind="ExternalInput")
    wvt = nc.dram_tensor("wvt", [E, A], BF16, kind="ExternalInput")
    bv = nc.dram_tensor("bv", [A], F32, kind="ExternalInput")
    identity = nc.dram_tensor("identity", [128, 128], BF16, kind="ExternalInput")
    out = nc.dram_tensor("out", [BLOC, A], F32, kind="ExternalOutput")

    with TileContext(nc, pool_alloc_mode="queue") as tc:
        _body(tc, enc, rc, crow, wvt, bv, identity, out)
    nc.finalize()
    return nc


_CACHE = {}


def _nc():
    if "nc" not in _CACHE:
        _CACHE["nc"] = _build()
    return _CACHE["nc"]


def _prep(encoder_outputs, decoder_hidden, Wq, bq, Wv, bv, Wd, bd):
    bf = ml_dtypes.bfloat16
    enc = np.ascontiguousarray(np.asarray(encoder_outputs, dtype=np.float32))
    h = np.asarray(decoder_hidden, dtype=np.float32)
    Wq = np.asarray(Wq, dtype=np.float32)
    bq = np.asarray(bq, dtype=np.float32)
    Wv = np.asarray(Wv, dtype=np.float32)
    bv = np.ascontiguousarray(np.asarray(bv, dtype=np.float32))
    Wd = np.asarray(Wd, dtype=np.float32)
    bd = np.asarray(bd, dtype=np.float32)

    # host-fused small projections (0.25% of the FLOPs):
    #   r = Wq.T (Wd h + bd) per batch; c = bq . (Wd h + bd)
    dec_q = h @ Wd.T + bd                   # [B, A]
    rv = dec_q @ Wq                         # [B, E]
    cv = dec_q @ bq                         # [B]

    wvt_b = np.ascontiguousarray(Wv.T.astype(bf))
    enc_b = enc.astype(bf)
    ident = np.ascontiguousarray(np.eye(128, dtype=np.float32).astype(bf))

    in_maps = []
    for i in range(NCORES):
        sl = slice(i * BLOC, (i + 1) * BLOC)
        in_maps.append({
            "enc": np.ascontiguousarray(enc_b[sl]),
            "rc": np.ascontiguousarray(rv[sl].astype(bf)),
            "crow": np.ascontiguousarray(cv[sl].reshape(1, BLOC)),
            "wvt": wvt_b,
            "bv": bv,
            "identity": ident,
        })
    return in_maps


def run(inputs, trace=False):
    in_maps = _prep(**inputs)
    res = run_bass_kernel_spmd(_nc(), in_maps, core_ids=list(range(NCORES)),
                               trace=trace)
    out = np.concatenate([r["out"] for r in res.results], axis=0).astype(np.float32)
    return out, res.exec_time_ns


def kernel(**inputs):
    out, _ = run(inputs, trace=False)
    return out


# revision 27
# speedup vs baseline: 1.0572x; 1.0021x over previous
"""Sparse cross-attention kernel for TRN2 (8 NeuronCores, SPMD data-parallel over batch).

Reference computation (per batch b):
    enc_q = enc @ Wq.T + bq; enc_v = enc @ Wv.T + bv; dec_q = Wd @ h + bd
    energy = tanh(enc_q @ dec_q); w = softmax(energy); out = w @ enc_v

Algebraic rewrite (exact, avoids materializing enc_q / enc_v):
    r[b] = Wq.T (Wd h_b + bd)               [E]   (tiny: computed on host, like
    c[b] = bq . (Wd h_b + bd)               scalar the host-fused GT=Wd.T Wq)
    energy[l] = enc[l,:] . r + c
    t    = tanh(energy) in [-1,1]  ->  exp() safe without max subtraction
    wexp = exp(t);  Z = sum_l wexp;  s~ = sum_l wexp[l] * enc[l,:]
    out  = (s~ @ Wv.T) / Z + bv             (1/Z + bias folded into one STT)

Turns a 210 GFLOP problem into a bf16 streaming problem bound by reading
encoder_outputs once (12.8 MB/core) + Wv.T (4 MB).

Device mapping per core (BLOC=16 batches, L split 128+68):
  The energy dot (mult + free-dim reduce over E) is split across engines by
  e-quarter units of [*,512]:
    DVE:   units (q0,lt0/1), (q1,lt0/1), (q2,lt0) as fused
           scalar_tensor_tensor with free-dim accumulate, reading the
           r-broadcast from PSUM (PE ones-matmul, produced a batch ahead).
    Pool:  units (q2,lt1), (q3,lt0/1) as tensor_tensor multiplies reading a
           DMA-broadcast r tile (HBM row -> 128 partitions, stride-0 AP).
    ACT:   reduces Pool's products via activation-accumulate; tanh; exp.
  PE:    r partition-broadcasts (PSUM, quarters 0-2), s~/Z accumulation via
         zero-padded-column lhsT (Z shares s's PSUM banks through a
         col-group-32 tile_position), s~ transposes, context matmuls.
  Tail:  drain s~ (ACT, chunked), transpose, context matmul vs Wv.T, then one
         STT fuses the 1/Z scale and +bv bias; single store.
"""

import numpy as np
import ml_dtypes

import concourse.bass as bass
import concourse.mybir as mybir
from concourse import bacc
from concourse.bass import ds
from concourse.tile import TileContext
from concourse.bass_utils import run_bass_kernel_spmd
from concourse._compat import with_exitstack

BF16 = mybir.dt.bfloat16
F32 = mybir.dt.float32

B, L, E, D, A = 128, 196, 2048, 1024, 1024
NCORES = 8
BLOC = B // NCORES          # 16 batches per core
EJ = E // 128               # 16 e-chunks of 128 (transposes / context)
NL0, NL1 = 128, L - 128     # l-tile sizes: 128 + 68


def _bcast(ap_row, parts):
    """AP reading a row-shaped DRAM slice broadcast across `parts` partitions."""
    inner = [list(x) for x in ap_row.ap]
    if len(inner) > 1 and inner[0][1] == 1:
        inner = inner[1:]
    return bass.AP(tensor=ap_row.tensor, offset=ap_row.offset,
                   ap=[[0, parts]] + inner)


@with_exitstack
def _body(ctx, tc, enc, rc, crow, wvt, bv, identity, out):
    nc = tc.nc
    AF = mybir.ActivationFunctionType
    OP = mybir.AluOpType

    consts = ctx.enter_context(tc.tile_pool(name="consts", bufs=1))

    ident = consts.tile([128, 128], BF16)
    ones_row = consts.tile([1, 128], BF16)    # lhsT for partition-broadcast matmul
    nc.vector.memset(ones_row[:, :], 1.0)
    ones_col = consts.tile([128, 1], BF16)    # rhs for the Z matmul
    nc.vector.memset(ones_col[:, :], 1.0)

    # r rows flattened onto partition 0 (matmul rhs must be base-partition 0)
    rc_flat = consts.tile([1, BLOC * E], BF16)
    nc.sync.dma_start(out=rc_flat[:, :], in_=rc[:, :])
    rcv = rc_flat.rearrange("p (b e) -> p b e", e=E)
    bv_rep = consts.tile([BLOC, A], F32)
    c_rep = consts.tile([128, BLOC], F32)
    # all-batch wexp lhsT: zeroed once; batch b writes its exp into col b
    wexp_all = consts.tile([128, BLOC, 2, BLOC], BF16)
    nc.vector.memset(wexp_all[:, :, :, :], 0.0)
    wvt_sb = consts.tile([128, EJ, A], BF16)

    # ---- loop pools ----
    rrep_ps_cm = tc.tile_pool(name="rrep_ps", bufs=4, space="PSUM")
    rrep_ps_pool = rrep_ps_cm.__enter__()
    rq_pool = ctx.enter_context(tc.tile_pool(name="rqp", bufs=3))
    enc_pool = ctx.enter_context(tc.tile_pool(name="encp", bufs=12))
    work = ctx.enter_context(tc.tile_pool(name="work", bufs=2))
    scr_pool = ctx.enter_context(tc.tile_pool(name="scr", bufs=6))
    prod_pool = ctx.enter_context(tc.tile_pool(name="prod", bufs=4))
    epi = ctx.enter_context(tc.tile_pool(name="epi", bufs=1))

    loop_psum_cm = tc.tile_pool(name="loop_psum", bufs=1, space="PSUM")
    loop_psum = loop_psum_cm.__enter__()
    # s~ split into two tiles so the two epilogue drains can run in
    # parallel; Z shares tile A's banks via the col-group-32 tile_position
    sz_psumA = loop_psum.tile([48, E // 2], F32)
    sz_psumB = loop_psum.tile([BLOC, E // 2], F32)
    s_psumA = sz_psumA[0:BLOC, :]
    s_psumB = sz_psumB[:, :]
    z_psum = sz_psumA[32:32 + BLOC, 0:1]

    def produce_rrep(b):
        """r[b, quarters 0-2] broadcast to PSUM via PE ones-matmuls (for the
        DVE STT units) + quarters 2-3 broadcast to SBUF bf16 via DMA (for the
        Pool tensor_tensor units; DVE DGE queue keeps SP free for enc)."""
        rps = []
        for q in range(3):
            rp = rrep_ps_pool.tile([128, 512], F32, tag="rrep_ps",
                                   name=f"rp{b}_{q}")
            nc.tensor.matmul(rp[:, :], ones_row[:, :],
                             rcv[0:1, b, ds(q * 512, 512)],
                             start=True, stop=True)
            rps.append(rp)
        rq = rq_pool.tile([128, 1024], BF16, tag="rq", name=f"rq{b}")
        nc.sync.dma_start(out=rq[:NL1, 0:512],
                          in_=_bcast(rc[b:b + 1, ds(E // 2, 512)], NL1))
        nc.sync.dma_start(out=rq[:, 512:1024],
                          in_=_bcast(rc[b:b + 1, ds(3 * E // 4, 512)], 128))
        return rps, rq

    def load_enc(b):
        et0 = enc_pool.tile([128, E], BF16, tag="enc", name=f"et0_{b}")
        nc.sync.dma_start(out=et0[:, :], in_=enc[b, 0:NL0, :])
        et1 = enc_pool.tile([128, E], BF16, tag="enc", name=f"et1_{b}")
        nc.sync.dma_start(out=et1[:NL1, :], in_=enc[b, NL0:L, :])
        return [(et0, NL0), (et1, NL1)]

    PF = 3   # enc prefetch depth (batches)
    # startup order matters: enc b0/b1 first on the SP queue; everything else
    # paced via tile_wait_until so dep-free DMAs can't starve the enc stream
    et_tiles = {0: load_enc(0)}
    rrep_tiles = {0: produce_rrep(0)}
    nc.sync.dma_start(out=c_rep[:, :], in_=_bcast(crow[0:1, :], 128))
    et_tiles[1] = load_enc(1)
    rrep_tiles[1] = produce_rrep(1)
    wvtv = wvt[:, :].rearrange("(j p) a -> p j a", p=128)
    et_tiles[2] = load_enc(2)
    pending = {}   # software pipeline: batch b's softmax+accumulate stage is
                   # emitted after batch b+1's energy stage

    def energy_stage(b):
        if b + 1 < BLOC and b + 1 not in rrep_tiles:
            rrep_tiles[b + 1] = produce_rrep(b + 1)
        if b + PF < BLOC:
            et_tiles[b + PF] = load_enc(b + PF)
        # one wvt chunk per batch on the same in-order SP queue, emitted
        # after the enc prefetch so it never delays the enc stream
        nc.sync.dma_start(out=wvt_sb[:, ds(b, 1), :],
                          in_=wvtv[:, ds(b, 1), :])
        if b == 1:
            nc.sync.dma_start(out=ident[:, :], in_=identity[:, :])
        if b == 2:
            nc.sync.dma_start(out=bv_rep[:, :], in_=_bcast(bv[:], BLOC))
        rrep_ps_b, rq_b = rrep_tiles.pop(b)
        ets = et_tiles.pop(b)

        # energy partial sums: separate tiles per l-tile so consecutive
        # units never touch the same tile (avoids false WAW serialization)
        esum0 = work.tile([128, 4], F32, tag="esum0", name=f"esum0_{b}")
        esum1 = work.tile([128, 5], F32, tag="esum1", name=f"esum1_{b}")
        esums = (esum0, esum1)
        # DVE fused units; for the last batch take the lt1 Pool units onto
        # DVE too (shorter tail critical path through Pool->ACT)
        dve_units = [(0, 0), (0, 1), (1, 0), (1, 1), (2, 0)]
        # (2, 1) with width 1024 = merged quarters 2+3 of l-tile 1 (their
        # partial sums share one esum slot; only the total matters)
        pool_units = [(2, 1, 1024), (3, 0, 512)]
        if b == BLOC - 1:
            dve_units += [(2, 1)]
            pool_units = [(3, 0, 512), (3, 1, 512)]
        elif b == 0:
            # all-DVE first batch: the softmax chain never waits on the cold
            # Pool->ACT path, so PE's s-matmuls and the next rreps flow early
            dve_units += [(2, 1), (3, 0), (3, 1)]
            pool_units = []
        SLIV = 128   # leading cols of (2,1) handled by DVE, rest by Pool
        for q, lt in dve_units:
            et, nl = ets[lt]
            scr = scr_pool.tile([128, 512], BF16, tag="scr",
                                name=f"scr{b}_{q}{lt}")
            in1 = (rrep_ps_b[q][:nl, :] if q < 3
                   else rq_b[:nl, ds((q - 2) * 512, 512)])
            nc.vector.scalar_tensor_tensor(
                out=scr[:nl, :],
                in0=et[:nl, ds(q * 512, 512)],
                scalar=0.0,
                in1=in1,
                op0=OP.bypass,
                op1=OP.mult,
                accum_out=esums[lt][:nl, ds(q, 1)],
            )
        # Pool multiplies (ACT reduces emitted separately, after the previous
        # batch's softmax, so tanh/exp never queue behind them on ACT)
        prods = []
        for q, lt, w in pool_units:
            et, nl = ets[lt]
            prod = prod_pool.tile([128, 1024], BF16, tag="prod",
                                  name=f"prod{b}_{q}{lt}")
            nc.gpsimd.tensor_tensor(
                out=prod[:nl, 0:w],
                in0=et[:nl, ds(q * 512, w)],
                in1=rq_b[:nl, ds((q - 2) * 512, w)],
                op=OP.mult)
            prods.append((q, lt, nl, w, prod))
        pending[b] = (ets, esums, prods)

    def act_reduces(b):
        ets, esums, prods = pending[b]
        for q, lt, nl, w, prod in prods:
            scr = scr_pool.tile([128, 1024], BF16, tag="ascr",
                                name=f"ascr{b}_{q}{lt}")
            nc.scalar.activation(out=scr[:nl, 0:w], in_=prod[:nl, 0:w],
                                 func=AF.Copy,
                                 accum_out=esums[lt][:nl, ds(q, 1)])

    def softmax_stage(b):
        if b >= BLOC - 2:
            # keep PE continuously busy so the tail matmuls run at full clock
            for w in range(4):
                fp = rrep_ps_pool.tile([128, 512], F32, tag="rrep_ps",
                                       name=f"warm{b}_{w}")
                nc.tensor.matmul(fp[:, :], ones_row[:, :],
                                 rcv[0:1, BLOC - 1, ds((w % 4) * 512, 512)],
                                 start=True, stop=True)
        ets, esums, prods = pending.pop(b)
        en = work.tile([128, 2], F32, tag="en", name=f"en{b}")
        enscr = work.tile([128, 2, 4], F32, tag="enscr", name=f"enscr{b}")
        for lt in range(2):
            # lt1 of steady batches has quarters 2+3 merged into slot 2;
            # slot 3 is unwritten there and must not be summed
            w = 3 if (lt == 1 and 0 < b < BLOC - 1) else 4
            nc.scalar.activation(out=enscr[:, lt, 0:w], in_=esums[lt][:, 0:w],
                                 func=AF.Copy, accum_out=en[:, ds(lt, 1)])
        tcol = work.tile([128, 2], F32, tag="tcol", name=f"tcol{b}")
        nc.scalar.activation(out=tcol[:, :], in_=en[:, :], func=AF.Tanh,
                             bias=c_rep[:, ds(b, 1)], scale=1.0)
        nc.scalar.activation(out=wexp_all[:, b, :, b], in_=tcol[:, :], func=AF.Exp)

        # s~ and Z accumulation across all batches (row b via zero-padded col b)
        for lt, (et, nl) in enumerate(ets):
            first = (b == 0 and lt == 0)
            last = (b == BLOC - 1 and lt == 1)
            wl = wexp_all[:nl, b, lt, :]
            nc.tensor.matmul(z_psum[:, :], wl, ones_col[:nl, :],
                             start=first, stop=last, tile_position=(0, 32))
            for j in range(4):
                dst = (s_psumA[:, ds(j * 512, 512)] if j < 2
                       else s_psumB[:, ds((j - 2) * 512, 512)])
                nc.tensor.matmul(
                    dst,
                    wl,
                    et[:nl, ds(j * 512, 512)],
                    start=first, stop=last,
                )

    for b in range(BLOC):
        energy_stage(b)
        if b >= 1:
            softmax_stage(b - 1)
        act_reduces(b)
    softmax_stage(BLOC - 1)

    # ---- epilogue: context = (s~ @ Wv.T) / Z + bv ----
    # drains paired DVE/ACT; transposes + copies + ctx interleaved per chunk
    # in drain-readiness order so PE rolls straight into the ctx matmuls
    s_sb = epi.tile([BLOC, E], BF16)
    nc.vector.tensor_copy(s_sb[:, 0:1024], s_psumA[:, :])
    nc.scalar.activation(out=s_sb[:, 1024:2048], in_=s_psumB[:, :],
                         func=AF.Copy)
    z_sb = epi.tile([BLOC, 1], F32)
    nc.vector.tensor_copy(z_sb[:, :], z_psum[:, :])
    zinv = epi.tile([BLOC, 1], F32)
    nc.vector.reciprocal(zinv[:, :], z_sb[:, :])
    loop_psum_cm.__exit__(None, None, None)
    rrep_ps_cm.__exit__(None, None, None)   # free banks for the epilogue

    sTs = [epi.tile([128, BLOC], BF16, name=f"sT{j}") for j in range(EJ)]
    cpool_cm = tc.tile_pool(name="ctx_psum", bufs=1, space="PSUM")
    cpool = cpool_cm.__enter__()
    ctx_ps = cpool.tile([BLOC, A], F32)
    tp_cm = tc.tile_pool(name="tp_psum", bufs=6, space="PSUM")
    tp_pool = tp_cm.__enter__()
    ctx_sb = epi.tile([BLOC, A], F32)

    def emit_transpose(j):
        tp = tp_pool.tile([128, BLOC], BF16, tag="tp")
        nc.tensor.transpose(tp[:, :], s_sb[:, ds(j * 128, 128)],
                            ident[:BLOC, :BLOC])
        if j % 2 == 1:
            nc.scalar.copy(sTs[j][:, :], tp[:, :])
        else:
            nc.vector.tensor_copy(sTs[j][:, :], tp[:, :])

    def emit_ctx(j, a2):
        nc.tensor.matmul(
            ctx_ps[:, ds(a2 * 512, 512)],
            sTs[j][:, :],
            wvt_sb[:, j, ds(a2 * 512, 512)],
            start=(j == 0), stop=(j == EJ - 1),
        )

    # transposes interleaved with the a0 context chain, offset so each ctx
    # matmul's sT copy has already landed (PE executes its stream in order)
    for j in range(8):
        emit_transpose(j)
    for j in range(8, EJ):
        emit_transpose(j)
        emit_ctx(j - 8, 0)
    for j in range(8, EJ):
        emit_ctx(j, 0)
    # a0 half done: normalize+bias+store it while PE runs the a1 half
    nc.vector.scalar_tensor_tensor(
        out=ctx_sb[:, 0:512], in0=ctx_ps[:, 0:512],
        scalar=zinv[:, 0:1], in1=bv_rep[:, 0:512],
        op0=OP.mult, op1=OP.add)
    nc.sync.dma_start(out=out[:, 0:512], in_=ctx_sb[:, 0:512])
    for j in range(EJ):
        emit_ctx(j, 1)
    nc.vector.scalar_tensor_tensor(
        out=ctx_sb[:, 512:1024], in0=ctx_ps[:, 512:1024],
        scalar=zinv[:, 0:1], in1=bv_rep[:, 512:1024],
        op0=OP.mult, op1=OP.add)
    nc.sync.dma_start(out=out[:, 512:1024], in_=ctx_sb[:, 512:1024])
    tp_cm.__exit__(None, None, None)
    cpool_cm.__exit__(None, None, None)


def _build():
    nc = bacc.Bacc()
    enc = nc.dram_tensor("enc", [BLOC, L, E], BF16, kind="ExternalInput")
    rc = nc.dram_tensor("rc", [BLOC, E], BF16, kind="ExternalInput")
    crow = nc.dram_tensor("crow", [1, BLOC], F32, kind="ExternalInput")
    wvt = nc.dram_tensor("wvt", [E, A], BF16, kind="ExternalInput")
    bv = nc.dram_tensor("bv", [A], F32, kind="ExternalInput")
    identity = nc.dram_tensor("identity", [128, 128], BF16, kind="ExternalInput")
    out = nc.dram_tensor("out", [BLOC, A], F32, kind="ExternalOutput")

    with TileContext(nc, pool_alloc_mode="queue") as tc:
        _body(tc, enc, rc, crow, wvt, bv, identity, out)
    nc.finalize()
    return nc


_CACHE = {}


def _nc():
    if "nc" not in _CACHE:
        _CACHE["nc"] = _build()
    return _CACHE["nc"]


def _prep(encoder_outputs, decoder_hidden, Wq, bq, Wv, bv, Wd, bd):
    bf = ml_dtypes.bfloat16
    enc = np.ascontiguousarray(np.asarray(encoder_outputs, dtype=np.float32))
    h = np.asarray(decoder_hidden, dtype=np.float32)
    Wq = np.asarray(Wq, dtype=np.float32)
    bq = np.asarray(bq, dtype=np.float32)
    Wv = np.asarray(Wv, dtype=np.float32)
    bv = np.ascontiguousarray(np.asarray(bv, dtype=np.float32))
    Wd = np.asarray(Wd, dtype=np.float32)
    bd = np.asarray(bd, dtype=np.float32)

    # host-fused small projections (0.25% of the FLOPs):
    #   r = Wq.T (Wd h + bd) per batch; c = bq . (Wd h + bd)
    dec_q = h @ Wd.T + bd                   # [B, A]
    rv = dec_q @ Wq                         # [B, E]
    cv = dec_q @ bq                         # [B]

    wvt_b = np.ascontiguousarray(Wv.T.astype(bf))
    enc_b = enc.astype(bf)
    ident = np.ascontiguousarray(np.eye(128, dtype=np.float32).astype(bf))

    in_maps = []
    for i in range(NCORES):
        sl = slice(i * BLOC, (i + 1) * BLOC)
        in_maps.append({
            "enc": np.ascontiguousarray(enc_b[sl]),
            "rc": np.ascontiguousarray(rv[sl].astype(bf)),
            "crow": np.ascontiguousarray(cv[sl].reshape(1, BLOC)),
            "wvt": wvt_b,
            "bv": bv,
            "identity": ident,
        })
    return in_maps


def run(inputs, trace=False):
    in_maps = _prep(**inputs)
    res = run_bass_kernel_spmd(_nc(), in_maps, core_ids=list(range(NCORES)),
                               trace=trace)
    out = np.concatenate([r["out"] for r in res.results], axis=0).astype(np.float32)
    return out, res.exec_time_ns


def kernel(**inputs):
    out, _ = run(inputs, trace=False)
    return out
